# revision 1
# baseline (speedup 1.0000x reference)
"""Multi-head dot-product attention (RoPE, causal) on 8 NeuronCores.

Sharding: data-parallel over batch (2) x tensor-parallel over heads (16 -> 4
per core). Each core projects q/k/v for its 4 heads, runs causal attention,
and computes a partial output projection; the host sums the 4 partials per
batch element.

Device layout notes:
- All matmul operands are float32r (TF32-like, full-rate on the PE for
  moving dims >= 256; every matmul here is emitted at width 512).
- Inputs are fed pre-transposed ([E, S]) so projections contract E on
  partitions; q/k are produced transposed per head ([D, S]).
- Scores are computed transposed (ST[s, t]) so the A@V contraction needs no
  on-chip transposes; softmax skips max-subtraction (scores are O(1) by
  construction). Causality is enforced with 4 precomputed additive mask
  variants (one per diagonal sub-block position) so all matmuls stay full
  width. The 1/rowsum lands after A@V: the denominator is computed by an
  all-ones [128,128] stationary matmul, which leaves the row-sum replicated
  across all partitions, so a single full-lane reciprocal + multiply
  normalizes during PSUM eviction.
- RoPE uses a de-interleaved head dim (even dims | odd dims), folded into a
  host-side permutation of Wq/Wk columns; scores are permutation-invariant.
"""

import numpy as np

B, S, E, N, D = 2, 2048, 2048, 16, 128
HL = 4           # local heads per core (8 cores = 2 batch x 4 head groups)
ND = HL * D      # 512
NT = S // 128    # 16 row tiles
NB = S // 512    # 4 row blocks
NE = E // 128    # 16 contraction tiles
MASK_VALUE = float(-0.7 * np.finfo(np.float32).max)

_NC_CACHE = {}


def _build_module():
    import concourse.bass as bass
    import concourse.mybir as mybir
    import concourse.tile as tile
    from concourse import bacc

    f32 = mybir.dt.float32
    f32r = mybir.dt.float32r
    Exp = mybir.ActivationFunctionType.Exp

    nc = bacc.Bacc("TRN2", target_bir_lowering=False, debug=False, num_devices=8)

    xq_d = nc.dram_tensor("xq_t", [E, S], f32, kind="ExternalInput").ap()
    xkv_d = nc.dram_tensor("xkv_t", [E, S], f32, kind="ExternalInput").ap()
    wq_d = nc.dram_tensor("wq", [E, ND], f32, kind="ExternalInput").ap()
    wk_d = nc.dram_tensor("wk", [E, ND], f32, kind="ExternalInput").ap()
    wv_d = nc.dram_tensor("wv", [E, ND], f32, kind="ExternalInput").ap()
    wo_d = nc.dram_tensor("wo", [ND, E], f32, kind="ExternalInput").ap()
    csd_d = nc.dram_tensor("csd", [128, S], f32, kind="ExternalInput").ap()
    sns_d = nc.dram_tensor("sns", [128, S], f32, kind="ExternalInput").ap()
    ones_d = nc.dram_tensor("ones", [128, 128], f32, kind="ExternalInput").ap()
    msk_d = nc.dram_tensor("msk", [128, 4 * 512], f32, kind="ExternalInput").ap()
    out_d = nc.dram_tensor("out", [S, E], f32, kind="ExternalOutput").ap()

    def load_w_grouped(pool, dram, tag):
        """[E, ND] weights as 4 tiles [128, 4*ND] (4 e-subtiles each)."""
        ws = []
        for eg in range(4):
            w = pool.tile([128, 4 * ND], f32r, tag=f"{tag}{eg}",
                          name=f"{tag}{eg}")
            nc.gpsimd.dma_start(
                w[:].rearrange("p (e n) -> p e n", e=4),
                dram[bass.ds(512 * eg, 512), :].bitcast(f32r)
                .rearrange("(e p) n -> p e n", p=128))
            ws.append(w)
        return ws

    def wslice(ws, et):
        return ws[et // 4][:, bass.ds(512 * (et % 4), 512)]

    with tile.TileContext(nc) as tc:
        with tc.tile_pool(name="qkp", bufs=1) as qk_pool, \
             tc.tile_pool(name="vp", bufs=1) as v_pool:
            qT = [qk_pool.tile([128, S], f32r, tag=f"qT{h}", name=f"qT{h}")
                  for h in range(HL)]
            kT = [qk_pool.tile([128, S], f32r, tag=f"kT{h}", name=f"kT{h}")
                  for h in range(HL)]
            vG = [v_pool.tile([128, 4 * ND], f32r, tag=f"vG{g}",
                              name=f"vG{g}") for g in range(4)]

            # ---- projections (RoPE tables live only here) ----
            with tc.tile_pool(name="tables", bufs=1) as tpool, \
                 tc.tile_pool(name="wkp", bufs=1) as wk_pool:
                csd = tpool.tile([128, S], f32, tag="csd")
                sns = tpool.tile([128, S], f32, tag="sns")

                def rope(dst, src_ps, tb, rope_pool):
                    tbs = bass.ts(tb, 512)
                    tmp = rope_pool.tile([128, 512], f32, tag="tmp",
                                         name="tmp")
                    nc.vector.tensor_mul(tmp[0:64, :], src_ps[64:128, :],
                                         sns[0:64, tbs])
                    nc.vector.tensor_mul(tmp[64:128, :], src_ps[0:64, :],
                                         sns[64:128, tbs])
                    nc.vector.tensor_mul(dst[:, tbs], src_ps[:],
                                         csd[:, tbs])
                    nc.vector.tensor_add(dst[:, tbs], dst[:, tbs], tmp[:])

                # ---- Q projection ----
                with nc.named_scope("proj_q"), \
                     tc.tile_pool(name="wqp", bufs=1) as wq_pool, \
                     tc.tile_pool(name="xq", bufs=3) as xpool, \
                     tc.tile_pool(name="qps", bufs=2, space="PSUM") as qps_pool, \
                     tc.tile_pool(name="rope", bufs=2) as rope_pool:
                    wq = load_w_grouped(wq_pool, wq_d, "wq")
                    nc.gpsimd.dma_start(csd[:], csd_d[:])
                    nc.gpsimd.dma_start(sns[:], sns_d[:])
                    wk = load_w_grouped(wk_pool, wk_d, "wk")
                    for tb in range(NB):
                        qps = [qps_pool.tile([128, 512], f32, tag=f"q{h}",
                                             name=f"qps{h}") for h in range(HL)]
                        for ep in range(NE // 2):  # e-tile pairs
                            x = xpool.tile([128, 2, 512], f32r, tag="x",
                                           name="x")
                            nc.sync.dma_start(
                                x[:],
                                xq_d[bass.ds(256 * ep, 256), bass.ts(tb, 512)]
                                .bitcast(f32r).rearrange("(e p) t -> p e t",
                                                         p=128))
                            for e2 in range(2):
                                et = 2 * ep + e2
                                for h in range(HL):
                                    nc.tensor.matmul(
                                        qps[h][:],
                                        wslice(wq, et)[:, bass.ts(h, 128)],
                                        x[:, e2], start=(et == 0),
                                        stop=(et == NE - 1))
                        for h in range(HL):
                            rope(qT[h], qps[h][:], tb, rope_pool)

                # ---- K + V projection ----
                with nc.named_scope("proj_kv"), \
                     tc.tile_pool(name="wvp", bufs=1) as wv_pool, \
                     tc.tile_pool(name="xkv", bufs=3) as xpool, \
                     tc.tile_pool(name="kps", bufs=1, space="PSUM") as kps_pool, \
                     tc.tile_pool(name="vps", bufs=1, space="PSUM") as vps_pool, \
                     tc.tile_pool(name="rope2", bufs=2) as rope_pool:
                    wv = load_w_grouped(wv_pool, wv_d, "wv")
                    for tb in range(NB):
                        kps = [kps_pool.tile([128, 512], f32, tag=f"k{h}",
                                             name=f"kps{h}") for h in range(HL)]
                        vps = [vps_pool.tile([128, ND], f32, tag=f"v{sv}",
                                             name=f"vps{sv}") for sv in range(4)]
                        for ep in range(NE // 2):
                            x = xpool.tile([128, 2, 512], f32r, tag="x",
                                           name="x")
                            nc.sync.dma_start(
                                x[:],
                                xkv_d[bass.ds(256 * ep, 256), bass.ts(tb, 512)]
                                .bitcast(f32r).rearrange("(e p) t -> p e t",
                                                         p=128))
                            for e2 in range(2):
                                et = 2 * ep + e2
                                for h in range(HL):
                                    nc.tensor.matmul(
                                        kps[h][:],
                                        wslice(wk, et)[:, bass.ts(h, 128)],
                                        x[:, e2], start=(et == 0),
                                        stop=(et == NE - 1))
                                for sv in range(4):
                                    nc.tensor.matmul(
                                        vps[sv][:], x[:, e2, bass.ts(sv, 128)],
                                        wslice(wv, et), start=(et == 0),
                                        stop=(et == NE - 1))
                        for h in range(HL):
                            rope(kT[h], kps[h][:], tb, rope_pool)
                        for sv in range(4):
                            nc.scalar.copy(vG[tb][:, bass.ts(sv, 512)],
                                           vps[sv][:])

            # ---- Attention ----
            with tc.tile_pool(name="uTp", bufs=1) as ut_pool:
                uT = [ut_pool.tile([128, S], f32r, tag=f"uT{h}", name=f"uT{h}")
                      for h in range(HL)]
                with nc.named_scope("attn"), \
                     tc.tile_pool(name="cst", bufs=1) as cpool, \
                     tc.tile_pool(name="et", bufs=1) as et_pool, \
                     tc.tile_pool(name="sps", bufs=2, space="PSUM") as sps_pool, \
                     tc.tile_pool(name="dps", bufs=2, space="PSUM") as dps_pool, \
                     tc.tile_pool(name="ups", bufs=2, space="PSUM") as ups_pool, \
                     tc.tile_pool(name="rcp", bufs=2) as rcp_pool:
                    ones = cpool.tile([128, 128], f32r, tag="ones")
                    msk = cpool.tile([128, 4 * 512], f32, tag="msk")
                    nc.gpsimd.dma_start(ones[:], ones_d[:].bitcast(f32r))
                    nc.gpsimd.dma_start(msk[:], msk_d[:])
                    eG = [et_pool.tile([128, 2048], f32r, tag=f"eG{g}",
                                       name=f"eG{g}") for g in range(4)]

                    def e_ap(si, w=512):
                        base = 512 * (si % 4)
                        return eG[si // 4][:, base:base + w]

                    for h in range(HL):
                        for tb in range(NB):
                            nsi = 4 * (tb + 1)
                            tbs = bass.ts(tb, 512)
                            for j in range(nsi // 2):  # s-tile pairs
                                sp = sps_pool.tile([128, 2, 512], f32,
                                                   tag="sp", name="sp")
                                for p2 in range(2):
                                    si = 2 * j + p2
                                    nc.tensor.matmul(
                                        sp[:, p2], kT[h][:, bass.ts(si, 128)],
                                        qT[h][:, tbs], start=True, stop=True)
                                    v = si - 4 * tb
                                    if v >= 0:
                                        nc.vector.tensor_add(
                                            sp[:, p2], sp[:, p2],
                                            msk[:, bass.ts(v, 512)])
                                nc.scalar.activation(
                                    eG[j // 2][:, bass.ts(j % 2, 1024)],
                                    sp[:].rearrange("p a b -> p (a b)"), Exp)
                            den = dps_pool.tile([128, 512], f32, tag="den",
                                                name="den")
                            for si in range(nsi):
                                nc.tensor.matmul(den[:], ones[:], e_ap(si),
                                                 start=(si == 0),
                                                 stop=(si == nsi - 1))
                            rec = rcp_pool.tile([128, 512], f32, tag="rec",
                                                name="rec")
                            nc.vector.reciprocal(rec[:], den[:])
                            up = ups_pool.tile([128, 512], f32, tag="up",
                                               name="up")
                            for si in range(nsi):
                                g, sv = si // 4, si % 4
                                nc.tensor.matmul(
                                    up[:],
                                    vG[g][:, 512 * sv + 128 * h:
                                          512 * sv + 128 * (h + 1)],
                                    e_ap(si), start=(si == 0),
                                    stop=(si == nsi - 1))
                            nc.vector.tensor_mul(uT[h][:, tbs], up[:], rec[:])

                # ---- Output projection ----
                with nc.named_scope("out_proj"), \
                     tc.tile_pool(name="wop", bufs=1) as wo_pool, \
                     tc.tile_pool(name="ops", bufs=2, space="PSUM") as ops_pool, \
                     tc.tile_pool(name="ob", bufs=3) as ob_pool:
                    wo = []
                    for h in range(HL):
                        w = wo_pool.tile([128, E], f32r, tag=f"wo{h}",
                                         name=f"wo{h}")
                        nc.gpsimd.dma_start(
                            w[:], wo_d[bass.ts(h, 128), :].bitcast(f32r))
                        wo.append(w)
                    for tt in range(NT):
                        op = ops_pool.tile([128, E], f32, tag="op", name="op")
                        for h in range(HL):
                            for ec in range(4):
                                nc.tensor.matmul(
                                    op[:, bass.ts(ec, 512)],
                                    uT[h][:, bass.ts(tt, 128)],
                                    wo[h][:, bass.ts(ec, 512)],
                                    start=(h == 0), stop=(h == HL - 1))
                        ob = ob_pool.tile([128, E], f32, tag="ob", name="ob")
                        nc.scalar.copy(ob[:], op[:])
                        nc.sync.dma_start(out_d[bass.ts(tt, 128), :], ob[:])

    nc.compile()
    return nc


def _get_module():
    if "nc" not in _NC_CACHE:
        _NC_CACHE["nc"] = _build_module()
    return _NC_CACHE["nc"]


def _host_prep(inputs_q, inputs_kv, positions, Wq, Wk, Wv, Wo):
    """Build the 8 per-core input maps."""
    perm = np.concatenate([np.arange(0, D, 2), np.arange(1, D, 2)])  # de-interleave
    scale = np.float32(1.0 / np.sqrt(D))
    half = D // 2
    timescale = 10000.0 ** (2.0 * np.arange(half, dtype=np.float64) / D)
    ones = np.ones((128, 128), dtype=np.float32)
    # mask variant v (diag sub-block at cols [128v, 128v+128)):
    # masked (additive MASK_VALUE) where col < 128*v + row
    s_i = np.arange(128)[:, None]
    c_i = np.arange(512)[None, :]
    msk = np.concatenate(
        [np.where(c_i < 128 * v + s_i, MASK_VALUE, 0.0) for v in range(4)],
        axis=1).astype(np.float32)

    in_maps = []
    for c in range(8):
        b = c // 4
        h0 = (c % 4) * HL
        angle = positions[b].astype(np.float64)[None, :] / timescale[:, None]  # [64,S]
        cs = np.cos(angle).astype(np.float32)
        sn = np.sin(angle).astype(np.float32)
        csd = np.concatenate([cs, cs], axis=0)               # [128, S]
        sns = np.concatenate([-sn, sn], axis=0)              # [128, S]
        wq = (Wq[:, h0:h0 + HL, :][:, :, perm] * scale).reshape(E, ND)
        wk = Wk[:, h0:h0 + HL, :][:, :, perm].reshape(E, ND)
        wv = Wv[:, h0:h0 + HL, :].reshape(E, ND)
        wo = Wo[h0:h0 + HL].reshape(ND, E)
        in_maps.append({
            "xq_t": np.ascontiguousarray(inputs_q[b].T),
            "xkv_t": np.ascontiguousarray(inputs_kv[b].T),
            "wq": np.ascontiguousarray(wq.astype(np.float32)),
            "wk": np.ascontiguousarray(wk.astype(np.float32)),
            "wv": np.ascontiguousarray(wv.astype(np.float32)),
            "wo": np.ascontiguousarray(wo.astype(np.float32)),
            "csd": csd, "sns": sns, "ones": ones, "msk": msk,
        })
    return in_maps


def kernel(inputs_q, inputs_kv, positions, Wq, Wk, Wv, Wo, _trace=False,
           _trace_kwargs=None):
    from concourse import bass_utils

    nc = _get_module()
    in_maps = _host_prep(inputs_q, inputs_kv, positions, Wq, Wk, Wv, Wo)
    res = bass_utils.run_bass_kernel_spmd(
        nc, in_maps, core_ids=list(range(8)), trace=_trace,
        **(_trace_kwargs or {}))
    if _trace:
        _NC_CACHE["last_results"] = res
    parts = [res.results[c]["out"] for c in range(8)]
    out0 = parts[0] + parts[1] + parts[2] + parts[3]
    out1 = parts[4] + parts[5] + parts[6] + parts[7]
    return np.stack([out0, out1]).astype(np.float32)



# revision 6
# speedup vs baseline: 1.2774x; 1.2774x over previous
"""Multi-head dot-product attention (RoPE, causal) on 8 NeuronCores.

Sharding: data-parallel over batch (2) x tensor-parallel over heads (16 -> 4
per core). Each core projects q/k/v for its 4 heads, runs causal attention,
and computes a partial output projection; the host sums the 4 partials per
batch element.

v2 design notes (vs the f32r baseline):
- All projection / score / denominator / A@V matmuls take bf16 operands
  (same PE rate as f32r at 512-wide, but half the DMA traffic and fast
  weight loads). Out-projection stays f32r (uT stationary, Wo moving).
- Causal mask is applied inside the scores matmul accumulation: a second
  128-wide matmul (identity stationary, triangle-mask moving) adds
  MASK_VALUE over the diagonal 128x128 triangle. Off-diagonal-masked
  columns of diagonal s-tiles are skipped entirely (scores/exp/den/AV all
  run on the live column range only).
- Softmax denominator comes from an all-ones stationary matmul (row sum
  replicated over partitions); 1/den uses reciprocal_approx_fast (~5x
  faster than the exact DVE reciprocal, ~18 bits).
- Attention is software-pipelined with a 1-block skew: PE issues scores of
  block i interleaved with den/AV of block i-1 (exp output double-buffered),
  so the Scalar-engine exp never stalls the PE. Out-projection matmuls for
  a row block are interleaved right after its last head, spreading the
  output DMA across the attention phase.
- RoPE uses a de-interleaved head dim (even dims | odd dims), folded into a
  host-side permutation of Wq/Wk columns; scores are permutation-invariant.
  K-rope runs from a bf16 SBUF stage (PSUM bank freed by a fast scalar
  copy); Q-rope reads PSUM directly (enough banks for double buffering).
"""

import numpy as np

B, S, E, N, D = 2, 2048, 2048, 16, 128
HL = 4           # local heads per core (8 cores = 2 batch x 4 head groups)
ND = HL * D      # 512
NT = S // 128    # 16 row tiles
NB = S // 512    # 4 row blocks
NE = E // 128    # 16 contraction tiles
MASK_VALUE = float(-0.7 * np.finfo(np.float32).max)

_NC_CACHE = {}


def _build_module():
    import concourse.bass as bass
    import concourse.mybir as mybir
    import concourse.tile as tile
    from concourse import bacc

    f32 = mybir.dt.float32
    f32r = mybir.dt.float32r
    bf16 = mybir.dt.bfloat16
    Exp = mybir.ActivationFunctionType.Exp

    nc = bacc.Bacc("TRN2", target_bir_lowering=False, debug=False, num_devices=8)

    xq_d = nc.dram_tensor("xq_t", [E, S], bf16, kind="ExternalInput").ap()
    xkv_d = nc.dram_tensor("xkv_t", [E, S], bf16, kind="ExternalInput").ap()
    wq_d = nc.dram_tensor("wq", [E, ND], bf16, kind="ExternalInput").ap()
    wk_d = nc.dram_tensor("wk", [E, ND], bf16, kind="ExternalInput").ap()
    wv_d = nc.dram_tensor("wv", [E, ND], bf16, kind="ExternalInput").ap()
    wo_d = nc.dram_tensor("wo", [ND, E], f32, kind="ExternalInput").ap()
    csd_d = nc.dram_tensor("csd", [128, S], bf16, kind="ExternalInput").ap()
    sns_d = nc.dram_tensor("sns", [128, S], bf16, kind="ExternalInput").ap()
    ones_d = nc.dram_tensor("ones", [128, 128], bf16, kind="ExternalInput").ap()
    eye_d = nc.dram_tensor("eye", [128, 128], bf16, kind="ExternalInput").ap()
    tri_d = nc.dram_tensor("tri", [128, 128], bf16, kind="ExternalInput").ap()
    out_d = nc.dram_tensor("out", [S, E], f32, kind="ExternalOutput").ap()

    def load_w_grouped(pool, dram, tag):
        """[E, ND] bf16 weights as 4 tiles [128, 4*ND] (4 e-subtiles each)."""
        ws = []
        for eg in range(4):
            w = pool.tile([128, 4 * ND], bf16, tag=f"{tag}{eg}",
                          name=f"{tag}{eg}")
            nc.gpsimd.dma_start(
                w[:].rearrange("p (e n) -> p e n", e=4),
                dram[bass.ds(512 * eg, 512), :]
                .rearrange("(e p) n -> p e n", p=128))
            ws.append(w)
        return ws

    def wslice(ws, et):
        return ws[et // 4][:, bass.ds(512 * (et % 4), 512)]

    with tile.TileContext(nc) as tc:
        with tc.tile_pool(name="qkp", bufs=1) as qk_pool, \
             tc.tile_pool(name="vp", bufs=1) as v_pool, \
             tc.tile_pool(name="wop", bufs=1) as wo_pool:
            qT = [qk_pool.tile([128, S], bf16, tag=f"qT{h}", name=f"qT{h}")
                  for h in range(HL)]
            kT = [qk_pool.tile([128, S], bf16, tag=f"kT{h}", name=f"kT{h}")
                  for h in range(HL)]
            vG = [v_pool.tile([128, 4 * ND], bf16, tag=f"vG{g}",
                              name=f"vG{g}") for g in range(4)]

            # ---- projections (RoPE tables live only here) ----
            with tc.tile_pool(name="tables", bufs=1) as tpool, \
                 tc.tile_pool(name="wkp", bufs=1) as wk_pool:
                csd = tpool.tile([128, S], bf16, tag="csd")
                sns = tpool.tile([128, S], bf16, tag="sns")

                def rope(dst, src, tb, rope_pool, tmp_dt):
                    tbs = bass.ts(tb, 512)
                    tmp = rope_pool.tile([128, 512], tmp_dt, tag="tmp",
                                         name="tmp")
                    nc.vector.tensor_mul(tmp[0:64, :], src[64:128, :],
                                         sns[0:64, tbs])
                    nc.vector.tensor_mul(tmp[64:128, :], src[0:64, :],
                                         sns[64:128, tbs])
                    nc.vector.tensor_mul(dst[:, tbs], src[:], csd[:, tbs])
                    nc.vector.tensor_add(dst[:, tbs], dst[:, tbs], tmp[:])

                # ---- Q projection ----
                with nc.named_scope("proj_q"), \
                     tc.tile_pool(name="wqp", bufs=1) as wq_pool, \
                     tc.tile_pool(name="xq", bufs=3) as xpool, \
                     tc.tile_pool(name="qps", bufs=2, space="PSUM") as qps_pool, \
                     tc.tile_pool(name="rope", bufs=2) as rope_pool:
                    wq = load_w_grouped(wq_pool, wq_d, "wq")
                    nc.gpsimd.dma_start(csd[:], csd_d[:])
                    nc.gpsimd.dma_start(sns[:], sns_d[:])
                    wk = load_w_grouped(wk_pool, wk_d, "wk")
                    for tb in range(NB):
                        qps = [qps_pool.tile([128, 512], f32, tag=f"q{h}",
                                             name=f"qps{h}") for h in range(HL)]
                        for ep in range(NE // 2):  # e-tile pairs
                            x = xpool.tile([128, 2, 512], bf16, tag="x",
                                           name="x")
                            nc.sync.dma_start(
                                x[:],
                                xq_d[bass.ds(256 * ep, 256), bass.ts(tb, 512)]
                                .rearrange("(e p) t -> p e t", p=128))
                            for e2 in range(2):
                                et = 2 * ep + e2
                                for h in range(HL):
                                    nc.tensor.matmul(
                                        qps[h][:],
                                        wslice(wq, et)[:, bass.ts(h, 128)],
                                        x[:, e2], start=(et == 0),
                                        stop=(et == NE - 1))
                        for h in range(HL):
                            rope(qT[h], qps[h][:], tb, rope_pool, f32)

                # ---- K + V projection ----
                with nc.named_scope("proj_kv"), \
                     tc.tile_pool(name="wvp", bufs=1) as wv_pool, \
                     tc.tile_pool(name="xkv", bufs=3) as xpool, \
                     tc.tile_pool(name="kps", bufs=1, space="PSUM") as kps_pool, \
                     tc.tile_pool(name="vps", bufs=1, space="PSUM") as vps_pool, \
                     tc.tile_pool(name="rope2", bufs=2) as rope_pool:
                    wv = load_w_grouped(wv_pool, wv_d, "wv")
                    # prefetch Wo for the out-projection (gpsimd queue, after wv)
                    wo = []
                    for h in range(HL):
                        w = wo_pool.tile([128, E], f32r, tag=f"wo{h}",
                                         name=f"wo{h}")
                        nc.gpsimd.dma_start(
                            w[:], wo_d[bass.ts(h, 128), :].bitcast(f32r))
                        wo.append(w)
                    for tb in range(NB):
                        kps = [kps_pool.tile([128, 512], f32, tag=f"k{h}",
                                             name=f"kps{h}") for h in range(HL)]
                        vps = [vps_pool.tile([128, ND], f32, tag=f"v{sv}",
                                             name=f"vps{sv}") for sv in range(4)]
                        for ep in range(NE // 2):
                            x = xpool.tile([128, 2, 512], bf16, tag="x",
                                           name="x")
                            nc.sync.dma_start(
                                x[:],
                                xkv_d[bass.ds(256 * ep, 256), bass.ts(tb, 512)]
                                .rearrange("(e p) t -> p e t", p=128))
                            for e2 in range(2):
                                et = 2 * ep + e2
                                for h in range(HL):
                                    nc.tensor.matmul(
                                        kps[h][:],
                                        wslice(wk, et)[:, bass.ts(h, 128)],
                                        x[:, e2], start=(et == 0),
                                        stop=(et == NE - 1))
                                for sv in range(4):
                                    nc.tensor.matmul(
                                        vps[sv][:], x[:, e2, bass.ts(sv, 128)],
                                        wslice(wv, et), start=(et == 0),
                                        stop=(et == NE - 1))
                        for h in range(HL):
                            rope(kT[h], kps[h][:], tb, rope_pool, f32)
                        for sv in range(4):
                            nc.scalar.copy(vG[tb][:, bass.ts(sv, 512)],
                                           vps[sv][:])

            # ---- Attention + out-projection, software-pipelined ----
            with nc.named_scope("attn"), \
                 tc.tile_pool(name="uTp", bufs=1) as ut_pool, \
                 tc.tile_pool(name="cst", bufs=1) as cpool, \
                 tc.tile_pool(name="et", bufs=1) as et_pool, \
                 tc.tile_pool(name="sps", bufs=3, space="PSUM") as sps_pool, \
                 tc.tile_pool(name="dps", bufs=1, space="PSUM") as dps_pool, \
                 tc.tile_pool(name="ups", bufs=2, space="PSUM") as ups_pool, \
                 tc.tile_pool(name="ops", bufs=2, space="PSUM") as ops_pool, \
                 tc.tile_pool(name="rcp", bufs=2) as rcp_pool, \
                 tc.tile_pool(name="ob", bufs=3) as ob_pool:
                uT = [ut_pool.tile([128, S], f32r, tag=f"uT{h}", name=f"uT{h}")
                      for h in range(HL)]
                ones = cpool.tile([128, 128], bf16, tag="ones")
                eye = cpool.tile([128, 128], bf16, tag="eye")
                tri = cpool.tile([128, 128], bf16, tag="tri")
                nc.gpsimd.dma_start(ones[:], ones_d[:])
                nc.gpsimd.dma_start(eye[:], eye_d[:])
                nc.gpsimd.dma_start(tri[:], tri_d[:])
                # exp tiles, double-buffered across pipeline generations
                eG = [[et_pool.tile([128, 2048], bf16, tag=f"eG{gen}{g}",
                                    name=f"eG{gen}{g}") for g in range(4)]
                      for gen in range(2)]

                def e_ap(gen, si, off=0):
                    base = 512 * (si % 4)
                    return eG[gen][si // 4][:, base + off:base + 512]

                blocks = [(tb, h) for tb in range(NB) for h in range(HL)]

                def live_off(tb, si):
                    """First live column (within the 512-wide t block) of
                    s-tile si; cols below it are fully masked."""
                    v = si - 4 * tb
                    return 128 * v if v > 0 else 0

                def sc_chunks(i):
                    """Scores + mask + exp for block i, one chunk per s-tile."""
                    tb, h = blocks[i]
                    gen = i % 2
                    nsi = 4 * (tb + 1)
                    chunks = []
                    for si in range(nsi):
                        def emit(si=si, tb=tb, h=h, gen=gen):
                            v = si - 4 * tb
                            off = live_off(tb, si)
                            sp = sps_pool.tile([128, 512], f32, tag="sp",
                                               name="sp")
                            nc.tensor.matmul(
                                sp[:, off:512], kT[h][:, bass.ts(si, 128)],
                                qT[h][:, 512 * tb + off:512 * (tb + 1)],
                                start=True, stop=(v < 0))
                            if v >= 0:
                                nc.tensor.matmul(
                                    sp[:, off:off + 128], eye[:], tri[:],
                                    start=False, stop=True)
                            nc.scalar.activation(e_ap(gen, si, off),
                                                 sp[:, off:512], Exp)
                        chunks.append(emit)
                    return chunks

                def da_chunks(i):
                    """Denominator, reciprocal, A@V, normalize for block i."""
                    tb, h = blocks[i]
                    gen = i % 2
                    nsi = 4 * (tb + 1)
                    state = {}

                    def start():
                        state["den"] = dps_pool.tile([128, 512], f32,
                                                     tag="den", name="den")
                        state["up"] = ups_pool.tile([128, 512], f32,
                                                    tag="up", name="up")
                        state["rec"] = rcp_pool.tile([128, 512], f32,
                                                     tag="rec", name="rec")
                    chunks = [start]
                    for si in range(nsi):
                        def emit(si=si, tb=tb, gen=gen):
                            off = live_off(tb, si)
                            nc.tensor.matmul(
                                state["den"][:, off:512], ones[:],
                                e_ap(gen, si, off), start=(si == 0),
                                stop=(si == nsi - 1))
                            if si == nsi - 1:
                                nc.vector.reciprocal_approx_fast(
                                    state["rec"][:], state["den"][:])
                        chunks.append(emit)
                    for si in range(nsi):
                        def emit(si=si, tb=tb, h=h, gen=gen):
                            g, sv = si // 4, si % 4
                            off = live_off(tb, si)
                            nc.tensor.matmul(
                                state["up"][:, off:512],
                                vG[g][:, 512 * sv + 128 * h:
                                      512 * sv + 128 * (h + 1)],
                                e_ap(gen, si, off), start=(si == 0),
                                stop=(si == nsi - 1))
                            if si == nsi - 1:
                                nc.vector.tensor_mul(
                                    uT[h][:, bass.ts(tb, 512)],
                                    state["up"][:], state["rec"][:])
                        chunks.append(emit)
                    return chunks

                def op_chunks(tb):
                    """Out-projection for row block tb (needs uT[*][tb])."""
                    chunks = []
                    for tt in range(4 * tb, 4 * tb + 4):
                        for ec in range(4):
                            def emit(tt=tt, ec=ec):
                                op = ops_pool.tile([128, 512], f32, tag="op",
                                                   name="op")
                                for h in range(HL):
                                    nc.tensor.matmul(
                                        op[:], uT[h][:, bass.ts(tt, 128)],
                                        wo[h][:, bass.ts(ec, 512)],
                                        start=(h == 0), stop=(h == HL - 1))
                                ob = ob_pool.tile([128, 512], f32, tag="ob",
                                                  name="ob")
                                nc.vector.tensor_copy(ob[:], op[:])
                                nc.sync.dma_start(
                                    out_d[bass.ts(tt, 128),
                                          bass.ds(512 * ec, 512)], ob[:])
                            chunks.append(emit)
                    return chunks

                def merge(a, b):
                    na, nb_ = len(a), len(b)
                    ia = ib = 0
                    while ia < na or ib < nb_:
                        if ib >= nb_ or (ia < na and ia * nb_ <= ib * na):
                            a[ia]()
                            ia += 1
                        else:
                            b[ib]()
                            ib += 1

                for i in range(len(blocks)):
                    sc = sc_chunks(i)
                    da = da_chunks(i - 1) if i > 0 else []
                    ptb, ph = blocks[i - 1] if i > 0 else (0, 0)
                    if i > 0 and ph == HL - 1:
                        da = da + op_chunks(ptb)
                    merge(sc, da)
                last = len(blocks) - 1
                for c in da_chunks(last) + op_chunks(blocks[last][0]):
                    c()

    nc.compile()
    return nc


def _get_module():
    if "nc" not in _NC_CACHE:
        _NC_CACHE["nc"] = _build_module()
    return _NC_CACHE["nc"]


def _host_prep(inputs_q, inputs_kv, positions, Wq, Wk, Wv, Wo):
    """Build the 8 per-core input maps."""
    import ml_dtypes
    bf16 = ml_dtypes.bfloat16

    perm = np.concatenate([np.arange(0, D, 2), np.arange(1, D, 2)])  # de-interleave
    scale = np.float32(1.0 / np.sqrt(D))
    half = D // 2
    timescale = 10000.0 ** (2.0 * np.arange(half, dtype=np.float64) / D)
    ones = np.ones((128, 128), dtype=bf16)
    eye = np.eye(128, dtype=np.float32).astype(bf16)
    s_i = np.arange(128)[:, None]
    c_i = np.arange(128)[None, :]
    tri = np.where(c_i < s_i, MASK_VALUE, 0.0).astype(bf16)

    in_maps = []
    for c in range(8):
        b = c // 4
        h0 = (c % 4) * HL
        angle = positions[b].astype(np.float64)[None, :] / timescale[:, None]  # [64,S]
        cs = np.cos(angle).astype(np.float32)
        sn = np.sin(angle).astype(np.float32)
        csd = np.concatenate([cs, cs], axis=0).astype(bf16)      # [128, S]
        sns = np.concatenate([-sn, sn], axis=0).astype(bf16)     # [128, S]
        wq = (Wq[:, h0:h0 + HL, :][:, :, perm] * scale).reshape(E, ND)
        wk = Wk[:, h0:h0 + HL, :][:, :, perm].reshape(E, ND)
        wv = Wv[:, h0:h0 + HL, :].reshape(E, ND)
        wo = Wo[h0:h0 + HL].reshape(ND, E)
        in_maps.append({
            "xq_t": np.ascontiguousarray(inputs_q[b].T).astype(bf16),
            "xkv_t": np.ascontiguousarray(inputs_kv[b].T).astype(bf16),
            "wq": np.ascontiguousarray(wq.astype(bf16)),
            "wk": np.ascontiguousarray(wk.astype(bf16)),
            "wv": np.ascontiguousarray(wv.astype(bf16)),
            "wo": np.ascontiguousarray(wo.astype(np.float32)),
            "csd": csd, "sns": sns, "ones": ones, "eye": eye, "tri": tri,
        })
    return in_maps


def kernel(inputs_q, inputs_kv, positions, Wq, Wk, Wv, Wo, _trace=False,
           _trace_kwargs=None):
    from concourse import bass_utils

    nc = _get_module()
    in_maps = _host_prep(inputs_q, inputs_kv, positions, Wq, Wk, Wv, Wo)
    res = bass_utils.run_bass_kernel_spmd(
        nc, in_maps, core_ids=list(range(8)), trace=_trace,
        **(_trace_kwargs or {}))
    if _trace:
        _NC_CACHE["last_results"] = res
    parts = [res.results[c]["out"] for c in range(8)]
    out0 = parts[0] + parts[1] + parts[2] + parts[3]
    out1 = parts[4] + parts[5] + parts[6] + parts[7]
    return np.stack([out0, out1]).astype(np.float32)


# revision 12
# speedup vs baseline: 1.3124x; 1.0274x over previous
"""Multi-head dot-product attention (RoPE, causal) on 8 NeuronCores.

Sharding: data-parallel over batch (2) x tensor-parallel over heads (16 -> 4
per core). Each core projects q/k/v for its 4 heads, runs causal attention,
and computes a partial output projection; the host sums the 4 partials per
batch element.

v2 design notes (vs the f32r baseline):
- All projection / score / denominator / A@V matmuls take bf16 operands
  (same PE rate as f32r at 512-wide, but half the DMA traffic and fast
  weight loads). Out-projection stays f32r (uT stationary, Wo moving).
- Causal mask is applied inside the scores matmul accumulation: a second
  128-wide matmul (identity stationary, triangle-mask moving) adds
  MASK_VALUE over the diagonal 128x128 triangle. Off-diagonal-masked
  columns of diagonal s-tiles are skipped entirely (scores/exp/den/AV all
  run on the live column range only).
- Softmax denominator comes from an all-ones stationary matmul (row sum
  replicated over partitions); 1/den uses reciprocal_approx_fast (~5x
  faster than the exact DVE reciprocal, ~18 bits).
- Attention is software-pipelined with a 1-block skew: PE issues scores of
  block i interleaved with den/AV of block i-1 (exp output double-buffered),
  so the Scalar-engine exp never stalls the PE. Out-projection matmuls for
  a row block are interleaved right after its last head, spreading the
  output DMA across the attention phase.
- RoPE uses a de-interleaved head dim (even dims | odd dims), folded into a
  host-side permutation of Wq/Wk columns; scores are permutation-invariant.
  K-rope runs from a bf16 SBUF stage (PSUM bank freed by a fast scalar
  copy); Q-rope reads PSUM directly (enough banks for double buffering).
"""

import numpy as np

B, S, E, N, D = 2, 2048, 2048, 16, 128
HL = 4           # local heads per core (8 cores = 2 batch x 4 head groups)
ND = HL * D      # 512
NT = S // 128    # 16 row tiles
NB = S // 512    # 4 row blocks
NE = E // 128    # 16 contraction tiles
MASK_VALUE = float(-0.7 * np.finfo(np.float32).max)

_NC_CACHE = {}


def _build_module():
    import concourse.bass as bass
    import concourse.mybir as mybir
    import concourse.tile as tile
    from concourse import bacc

    f32 = mybir.dt.float32
    f32r = mybir.dt.float32r
    bf16 = mybir.dt.bfloat16
    Exp = mybir.ActivationFunctionType.Exp

    nc = bacc.Bacc("TRN2", target_bir_lowering=False, debug=False, num_devices=8)

    xq_d = nc.dram_tensor("xq_t", [E, S], bf16, kind="ExternalInput").ap()
    xkv_d = nc.dram_tensor("xkv_t", [E, S], bf16, kind="ExternalInput").ap()
    wq_d = nc.dram_tensor("wq", [E, ND], bf16, kind="ExternalInput").ap()
    wk_d = nc.dram_tensor("wk", [E, ND], bf16, kind="ExternalInput").ap()
    wv_d = nc.dram_tensor("wv", [E, ND], bf16, kind="ExternalInput").ap()
    wo_d_bf = nc.dram_tensor("wo", [ND, E], bf16, kind="ExternalInput").ap()
    csd_d = nc.dram_tensor("csd", [128, S], bf16, kind="ExternalInput").ap()
    sns_d = nc.dram_tensor("sns", [128, S], bf16, kind="ExternalInput").ap()
    ones_d = nc.dram_tensor("ones", [128, 128], bf16, kind="ExternalInput").ap()
    eye_d = nc.dram_tensor("eye", [128, 128], bf16, kind="ExternalInput").ap()
    tri_d = nc.dram_tensor("tri", [128, 128], bf16, kind="ExternalInput").ap()
    out_d = nc.dram_tensor("out", [S, E], f32, kind="ExternalOutput").ap()

    def load_w_grouped(pool, dram, tag, engine=None):
        """[E, ND] bf16 weights as 4 tiles [128, 4*ND] (4 e-subtiles each)."""
        eng = engine or nc.gpsimd
        ws = []
        for eg in range(4):
            w = pool.tile([128, 4 * ND], bf16, tag=f"{tag}{eg}",
                          name=f"{tag}{eg}")
            eng.dma_start(
                w[:].rearrange("p (e n) -> p e n", e=4),
                dram[bass.ds(512 * eg, 512), :]
                .rearrange("(e p) n -> p e n", p=128))
            ws.append(w)
        return ws

    def wslice(ws, et):
        return ws[et // 4][:, bass.ds(512 * (et % 4), 512)]

    with tile.TileContext(nc) as tc:
        with tc.tile_pool(name="qkp", bufs=1) as qk_pool, \
             tc.tile_pool(name="vp", bufs=1) as v_pool, \
             tc.tile_pool(name="wop", bufs=1) as wo_pool, \
             tc.tile_pool(name="cst", bufs=1) as cpool:
            qT = [qk_pool.tile([128, S], bf16, tag=f"qT{h}", name=f"qT{h}")
                  for h in range(HL)]
            kT = [qk_pool.tile([128, S], bf16, tag=f"kT{h}", name=f"kT{h}")
                  for h in range(HL)]
            vG = [v_pool.tile([128, 4 * ND], bf16, tag=f"vG{g}",
                              name=f"vG{g}") for g in range(4)]
            # tiny constants first on the gpsimd queue (96 KB)
            ones = cpool.tile([128, 128], bf16, tag="ones")
            eye = cpool.tile([128, 128], bf16, tag="eye")
            tri = cpool.tile([128, 128], bf16, tag="tri")
            nc.gpsimd.dma_start(ones[:], ones_d[:])
            nc.gpsimd.dma_start(eye[:], eye_d[:])
            nc.gpsimd.dma_start(tri[:], tri_d[:])

            # ---- projections (RoPE tables live only here) ----
            with tc.tile_pool(name="tables", bufs=1) as tpool, \
                 tc.tile_pool(name="wkp", bufs=1) as wk_pool:
                csd = tpool.tile([128, S], bf16, tag="csd")
                sns = tpool.tile([128, S], bf16, tag="sns")

                def rope(dst, src, tb, rope_pool, tmp_dt):
                    tbs = bass.ts(tb, 512)
                    tmp = rope_pool.tile([128, 512], tmp_dt, tag="tmp",
                                         name="tmp")
                    nc.vector.tensor_mul(tmp[0:64, :], src[64:128, :],
                                         sns[0:64, tbs])
                    nc.vector.tensor_mul(tmp[64:128, :], src[0:64, :],
                                         sns[64:128, tbs])
                    nc.vector.tensor_mul(dst[:, tbs], src[:], csd[:, tbs])
                    nc.vector.tensor_add(dst[:, tbs], dst[:, tbs], tmp[:])

                with tc.tile_pool(name="xp", bufs=5) as xpool:
                    # ---- Q projection ----
                    with nc.named_scope("proj_q"), \
                         tc.tile_pool(name="wqp", bufs=1) as wq_pool, \
                         tc.tile_pool(name="wvp", bufs=1) as wv_pool, \
                         tc.tile_pool(name="qps", bufs=2, space="PSUM") as qps_pool, \
                         tc.tile_pool(name="rope", bufs=2) as rope_pool:
                        wq = load_w_grouped(wq_pool, wq_d, "wq")
                        nc.gpsimd.dma_start(csd[:], csd_d[:])
                        nc.gpsimd.dma_start(sns[:], sns_d[:])
                        wk = wv = wo = None
                        for tb in range(NB):
                            qps = [qps_pool.tile([128, 512], f32, tag=f"q{h}",
                                                 name=f"qps{h}")
                                   for h in range(HL)]
                            for ep in range(NE // 2):  # e-tile pairs
                                x = xpool.tile([128, 2, 512], bf16, tag="x",
                                               name="x")
                                nc.sync.dma_start(
                                    x[:],
                                    xq_d[bass.ds(256 * ep, 256),
                                         bass.ts(tb, 512)]
                                    .rearrange("(e p) t -> p e t", p=128))
                                for e2 in range(2):
                                    et = 2 * ep + e2
                                    for h in range(HL):
                                        nc.tensor.matmul(
                                            qps[h][:],
                                            wslice(wq, et)[:, bass.ts(h, 128)],
                                            x[:, e2], start=(et == 0),
                                            stop=(et == NE - 1))
                            if tb == 0:
                                # deferred weight prefetch on the sync queue,
                                # sequenced between x-tile streams
                                wk = load_w_grouped(wk_pool, wk_d, "wk",
                                                    engine=nc.sync)
                            if tb == 2:
                                wv = load_w_grouped(wv_pool, wv_d, "wv",
                                                    engine=nc.sync)
                            for h in range(HL):
                                rope(qT[h], qps[h][:], tb, rope_pool, f32)

                    # ---- K + V projection ----
                    with nc.named_scope("proj_kv"), \
                         tc.tile_pool(name="kst", bufs=3) as kst_pool, \
                         tc.tile_pool(name="kps", bufs=1, space="PSUM") as kps_pool, \
                         tc.tile_pool(name="vps", bufs=1, space="PSUM") as vps_pool, \
                         tc.tile_pool(name="rope2", bufs=2) as rope_pool:
                        for tb in range(NB):
                            kps = [kps_pool.tile([128, 512], f32, tag=f"k{h}",
                                                 name=f"kps{h}")
                                   for h in range(HL)]
                            vps = [vps_pool.tile([128, ND], f32, tag=f"v{sv}",
                                                 name=f"vps{sv}")
                                   for sv in range(4)]
                            for ep in range(NE // 2):
                                x = xpool.tile([128, 2, 512], bf16, tag="x",
                                               name="x")
                                nc.sync.dma_start(
                                    x[:],
                                    xkv_d[bass.ds(256 * ep, 256),
                                          bass.ts(tb, 512)]
                                    .rearrange("(e p) t -> p e t", p=128))
                                for e2 in range(2):
                                    et = 2 * ep + e2
                                    for h in range(HL):
                                        nc.tensor.matmul(
                                            kps[h][:],
                                            wslice(wk, et)[:, bass.ts(h, 128)],
                                            x[:, e2], start=(et == 0),
                                            stop=(et == NE - 1))
                                    for sv in range(4):
                                        nc.tensor.matmul(
                                            vps[sv][:],
                                            x[:, e2, bass.ts(sv, 128)],
                                            wslice(wv, et), start=(et == 0),
                                            stop=(et == NE - 1))
                            if tb == 0:
                                # Wo prefetch on the sync queue (used by the
                                # out-projection interleaved into attention)
                                wo = []
                                for h in range(HL):
                                    w = wo_pool.tile([128, E], bf16,
                                                     tag=f"wo{h}",
                                                     name=f"wo{h}")
                                    nc.sync.dma_start(
                                        w[:], wo_d_bf[bass.ts(h, 128), :])
                                    wo.append(w)
                            for h in range(HL):
                                # hybrid rope: the half-swapped products read
                                # PSUM (partition crossbar OK), the aligned
                                # tail reads a bf16 SBUF stage so the PSUM
                                # bank is released early
                                tbs = bass.ts(tb, 512)
                                tmp = rope_pool.tile([128, 512], f32,
                                                     tag="tmp", name="tmp")
                                kst = kst_pool.tile([128, 512], bf16,
                                                    tag="kst", name="kst")
                                nc.vector.tensor_mul(tmp[0:64, :],
                                                     kps[h][64:128, :],
                                                     sns[0:64, tbs])
                                nc.vector.tensor_mul(tmp[64:128, :],
                                                     kps[h][0:64, :],
                                                     sns[64:128, tbs])
                                nc.scalar.copy(kst[:], kps[h][:])
                                nc.vector.tensor_mul(kT[h][:, tbs], kst[:],
                                                     csd[:, tbs])
                                nc.vector.tensor_add(kT[h][:, tbs],
                                                     kT[h][:, tbs], tmp[:])
                            for sv in range(4):
                                nc.scalar.copy(vG[tb][:, bass.ts(sv, 512)],
                                               vps[sv][:])

            # ---- Attention + out-projection, software-pipelined ----
            # PSUM pool order matters: up/op land on banks freed by the
            # (slow, DVE-bound) K-rope of the last block; sps/den land on
            # banks freed by the fast scalar V-evictions — so attention can
            # start before the K-rope tail drains.
            with nc.named_scope("attn"), \
                 tc.tile_pool(name="uTp", bufs=1) as ut_pool, \
                 tc.tile_pool(name="et", bufs=1) as et_pool, \
                 tc.tile_pool(name="ups", bufs=2, space="PSUM") as ups_pool, \
                 tc.tile_pool(name="ops", bufs=2, space="PSUM") as ops_pool, \
                 tc.tile_pool(name="sps", bufs=3, space="PSUM") as sps_pool, \
                 tc.tile_pool(name="dps", bufs=1, space="PSUM") as dps_pool, \
                 tc.tile_pool(name="rcp", bufs=2) as rcp_pool, \
                 tc.tile_pool(name="ob", bufs=3) as ob_pool:
                uT = [ut_pool.tile([128, S], bf16, tag=f"uT{h}", name=f"uT{h}")
                      for h in range(HL)]
                # exp tiles, double-buffered across pipeline generations
                eG = [[et_pool.tile([128, 2048], bf16, tag=f"eG{gen}{g}",
                                    name=f"eG{gen}{g}") for g in range(4)]
                      for gen in range(2)]

                def e_ap(gen, si, off=0):
                    base = 512 * (si % 4)
                    return eG[gen][si // 4][:, base + off:base + 512]

                blocks = [(tb, h) for tb in range(NB) for h in range(HL)]

                def live_off(tb, si):
                    """First live column (within the 512-wide t block) of
                    s-tile si; cols below it are fully masked."""
                    v = si - 4 * tb
                    return 128 * v if v > 0 else 0

                def sc_chunks(i):
                    """Scores + mask + exp for block i, one chunk per s-tile."""
                    tb, h = blocks[i]
                    gen = i % 2
                    nsi = 4 * (tb + 1)
                    chunks = []
                    for si in range(nsi):
                        def emit(si=si, tb=tb, h=h, gen=gen):
                            v = si - 4 * tb
                            off = live_off(tb, si)
                            sp = sps_pool.tile([128, 512], f32, tag="sp",
                                               name="sp")
                            nc.tensor.matmul(
                                sp[:, off:512], kT[h][:, bass.ts(si, 128)],
                                qT[h][:, 512 * tb + off:512 * (tb + 1)],
                                start=True, stop=(v < 0))
                            if v >= 0:
                                nc.tensor.matmul(
                                    sp[:, off:off + 128], eye[:], tri[:],
                                    start=False, stop=True)
                            nc.scalar.activation(e_ap(gen, si, off),
                                                 sp[:, off:512], Exp)
                        chunks.append(emit)
                    return chunks

                def da_chunks(i):
                    """Denominator, reciprocal, A@V, normalize for block i."""
                    tb, h = blocks[i]
                    gen = i % 2
                    nsi = 4 * (tb + 1)
                    state = {}

                    def start():
                        state["den"] = dps_pool.tile([128, 512], f32,
                                                     tag="den", name="den")
                        state["up"] = ups_pool.tile([128, 512], f32,
                                                    tag="up", name="up")
                        state["rec"] = rcp_pool.tile([128, 512], f32,
                                                     tag="rec", name="rec")
                    chunks = [start]
                    for si in range(nsi):
                        def emit(si=si, tb=tb, gen=gen):
                            off = live_off(tb, si)
                            nc.tensor.matmul(
                                state["den"][:, off:512], ones[:],
                                e_ap(gen, si, off), start=(si == 0),
                                stop=(si == nsi - 1))
                            if si == nsi - 1:
                                nc.vector.reciprocal_approx_fast(
                                    state["rec"][:], state["den"][:])
                        chunks.append(emit)
                    for si in range(nsi):
                        def emit(si=si, tb=tb, h=h, gen=gen):
                            g, sv = si // 4, si % 4
                            off = live_off(tb, si)
                            nc.tensor.matmul(
                                state["up"][:, off:512],
                                vG[g][:, 512 * sv + 128 * h:
                                      512 * sv + 128 * (h + 1)],
                                e_ap(gen, si, off), start=(si == 0),
                                stop=(si == nsi - 1))
                            if si == nsi - 1:
                                nc.vector.tensor_mul(
                                    uT[h][:, bass.ts(tb, 512)],
                                    state["up"][:], state["rec"][:])
                        chunks.append(emit)
                    return chunks

                def op_chunks(tb):
                    """Out-projection for row block tb (needs uT[*][tb])."""
                    chunks = []
                    for tt in range(4 * tb, 4 * tb + 4):
                        for ec in range(4):
                            def emit(tt=tt, ec=ec):
                                op = ops_pool.tile([128, 512], f32, tag="op",
                                                   name="op")
                                for h in range(HL):
                                    nc.tensor.matmul(
                                        op[:], uT[h][:, bass.ts(tt, 128)],
                                        wo[h][:, bass.ts(ec, 512)],
                                        start=(h == 0), stop=(h == HL - 1))
                                ob = ob_pool.tile([128, 512], f32, tag="ob",
                                                  name="ob")
                                nc.vector.tensor_copy(ob[:], op[:])
                                nc.sync.dma_start(
                                    out_d[bass.ts(tt, 128),
                                          bass.ds(512 * ec, 512)], ob[:])
                            chunks.append(emit)
                    return chunks

                def merge(a, b):
                    na, nb_ = len(a), len(b)
                    ia = ib = 0
                    while ia < na or ib < nb_:
                        if ib >= nb_ or (ia < na and ia * nb_ <= ib * na):
                            a[ia]()
                            ia += 1
                        else:
                            b[ib]()
                            ib += 1

                for i in range(len(blocks)):
                    sc = sc_chunks(i)
                    da = da_chunks(i - 1) if i > 0 else []
                    ptb, ph = blocks[i - 1] if i > 0 else (0, 0)
                    if i > 0 and ph == HL - 1:
                        da = da + op_chunks(ptb)
                    merge(sc, da)
                last = len(blocks) - 1
                for c in da_chunks(last) + op_chunks(blocks[last][0]):
                    c()

    nc.compile()
    return nc


def _get_module():
    if "nc" not in _NC_CACHE:
        _NC_CACHE["nc"] = _build_module()
    return _NC_CACHE["nc"]


def _host_prep(inputs_q, inputs_kv, positions, Wq, Wk, Wv, Wo):
    """Build the 8 per-core input maps."""
    import ml_dtypes
    bf16 = ml_dtypes.bfloat16

    perm = np.concatenate([np.arange(0, D, 2), np.arange(1, D, 2)])  # de-interleave
    scale = np.float32(1.0 / np.sqrt(D))
    half = D // 2
    timescale = 10000.0 ** (2.0 * np.arange(half, dtype=np.float64) / D)
    ones = np.ones((128, 128), dtype=bf16)
    eye = np.eye(128, dtype=np.float32).astype(bf16)
    s_i = np.arange(128)[:, None]
    c_i = np.arange(128)[None, :]
    tri = np.where(c_i < s_i, MASK_VALUE, 0.0).astype(bf16)

    in_maps = []
    for c in range(8):
        b = c // 4
        h0 = (c % 4) * HL
        angle = positions[b].astype(np.float64)[None, :] / timescale[:, None]  # [64,S]
        cs = np.cos(angle).astype(np.float32)
        sn = np.sin(angle).astype(np.float32)
        csd = np.concatenate([cs, cs], axis=0).astype(bf16)      # [128, S]
        sns = np.concatenate([-sn, sn], axis=0).astype(bf16)     # [128, S]
        wq = (Wq[:, h0:h0 + HL, :][:, :, perm] * scale).reshape(E, ND)
        wk = Wk[:, h0:h0 + HL, :][:, :, perm].reshape(E, ND)
        wv = Wv[:, h0:h0 + HL, :].reshape(E, ND)
        wo = Wo[h0:h0 + HL].reshape(ND, E)
        in_maps.append({
            "xq_t": np.ascontiguousarray(inputs_q[b].T).astype(bf16),
            "xkv_t": np.ascontiguousarray(inputs_kv[b].T).astype(bf16),
            "wq": np.ascontiguousarray(wq.astype(bf16)),
            "wk": np.ascontiguousarray(wk.astype(bf16)),
            "wv": np.ascontiguousarray(wv.astype(bf16)),
            "wo": np.ascontiguousarray(wo.astype(bf16)),
            "csd": csd, "sns": sns, "ones": ones, "eye": eye, "tri": tri,
        })
    return in_maps


def kernel(inputs_q, inputs_kv, positions, Wq, Wk, Wv, Wo, _trace=False,
           _trace_kwargs=None):
    from concourse import bass_utils

    nc = _get_module()
    in_maps = _host_prep(inputs_q, inputs_kv, positions, Wq, Wk, Wv, Wo)
    res = bass_utils.run_bass_kernel_spmd(
        nc, in_maps, core_ids=list(range(8)), trace=_trace,
        **(_trace_kwargs or {}))
    if _trace:
        _NC_CACHE["last_results"] = res
    parts = [res.results[c]["out"] for c in range(8)]
    out0 = parts[0] + parts[1] + parts[2] + parts[3]
    out1 = parts[4] + parts[5] + parts[6] + parts[7]
    return np.stack([out0, out1]).astype(np.float32)


# revision 19
# speedup vs baseline: 1.3163x; 1.0030x over previous
"""Multi-head dot-product attention (RoPE, causal) on 8 NeuronCores.

Sharding: data-parallel over batch (2) x tensor-parallel over heads (16 -> 4
per core). Each core projects q/k/v for its 4 heads, runs causal attention,
and computes a partial output projection; the host sums the 4 partials per
batch element.

v2 design notes (vs the f32r baseline):
- All projection / score / denominator / A@V matmuls take bf16 operands
  (same PE rate as f32r at 512-wide, but half the DMA traffic and fast
  weight loads). Out-projection stays f32r (uT stationary, Wo moving).
- Causal mask is applied inside the scores matmul accumulation: a second
  128-wide matmul (identity stationary, triangle-mask moving) adds
  MASK_VALUE over the diagonal 128x128 triangle. Off-diagonal-masked
  columns of diagonal s-tiles are skipped entirely (scores/exp/den/AV all
  run on the live column range only).
- Softmax denominator comes from an all-ones stationary matmul (row sum
  replicated over partitions); 1/den uses reciprocal_approx_fast (~5x
  faster than the exact DVE reciprocal, ~18 bits).
- Attention is software-pipelined with a 1-block skew: PE issues scores of
  block i interleaved with den/AV of block i-1 (exp output double-buffered),
  so the Scalar-engine exp never stalls the PE. Out-projection matmuls for
  a row block are interleaved right after its last head, spreading the
  output DMA across the attention phase.
- RoPE uses a de-interleaved head dim (even dims | odd dims), folded into a
  host-side permutation of Wq/Wk columns; scores are permutation-invariant.
  K-rope runs from a bf16 SBUF stage (PSUM bank freed by a fast scalar
  copy); Q-rope reads PSUM directly (enough banks for double buffering).
"""

import numpy as np

B, S, E, N, D = 2, 2048, 2048, 16, 128
HL = 4           # local heads per core (8 cores = 2 batch x 4 head groups)
ND = HL * D      # 512
NT = S // 128    # 16 row tiles
NB = S // 512    # 4 row blocks
NE = E // 128    # 16 contraction tiles
MASK_VALUE = float(-0.7 * np.finfo(np.float32).max)

_NC_CACHE = {}


def _build_module():
    import concourse.bass as bass
    import concourse.mybir as mybir
    import concourse.tile as tile
    from concourse import bacc

    f32 = mybir.dt.float32
    f32r = mybir.dt.float32r
    bf16 = mybir.dt.bfloat16
    Exp = mybir.ActivationFunctionType.Exp

    nc = bacc.Bacc("TRN2", target_bir_lowering=False, debug=False, num_devices=8)

    xq_d = nc.dram_tensor("xq_t", [E, S], bf16, kind="ExternalInput").ap()
    xkv_d = nc.dram_tensor("xkv_t", [E, S], bf16, kind="ExternalInput").ap()
    wq_d = nc.dram_tensor("wq", [E, ND], bf16, kind="ExternalInput").ap()
    wk_d = nc.dram_tensor("wk", [E, ND], bf16, kind="ExternalInput").ap()
    wv_d = nc.dram_tensor("wv", [E, ND], bf16, kind="ExternalInput").ap()
    wo_d_bf = nc.dram_tensor("wo", [ND, E], bf16, kind="ExternalInput").ap()
    csd_d = nc.dram_tensor("csd", [128, S], bf16, kind="ExternalInput").ap()
    sns_d = nc.dram_tensor("sns", [128, S], bf16, kind="ExternalInput").ap()
    ones_d = nc.dram_tensor("ones", [128, 128], bf16, kind="ExternalInput").ap()
    eye_d = nc.dram_tensor("eye", [128, 128], bf16, kind="ExternalInput").ap()
    tri_d = nc.dram_tensor("tri", [128, 128], bf16, kind="ExternalInput").ap()
    out_d = nc.dram_tensor("out", [S, E], f32, kind="ExternalOutput").ap()

    def load_w_grouped(pool, dram, tag, engine=None):
        """[E, ND] bf16 weights as 4 tiles [128, 4*ND] (4 e-subtiles each)."""
        eng = engine or nc.gpsimd
        ws = []
        for eg in range(4):
            w = pool.tile([128, 4 * ND], bf16, tag=f"{tag}{eg}",
                          name=f"{tag}{eg}")
            eng.dma_start(
                w[:].rearrange("p (e n) -> p e n", e=4),
                dram[bass.ds(512 * eg, 512), :]
                .rearrange("(e p) n -> p e n", p=128))
            ws.append(w)
        return ws

    def wslice(ws, et):
        return ws[et // 4][:, bass.ds(512 * (et % 4), 512)]

    with tile.TileContext(nc) as tc:
        with tc.tile_pool(name="qkp", bufs=1) as qk_pool, \
             tc.tile_pool(name="vp", bufs=1) as v_pool, \
             tc.tile_pool(name="wop", bufs=1) as wo_pool, \
             tc.tile_pool(name="cst", bufs=1) as cpool:
            qT = [qk_pool.tile([128, S], bf16, tag=f"qT{h}", name=f"qT{h}")
                  for h in range(HL)]
            kT = [qk_pool.tile([128, S], bf16, tag=f"kT{h}", name=f"kT{h}")
                  for h in range(HL)]
            vG = [v_pool.tile([128, 4 * ND], bf16, tag=f"vG{g}",
                              name=f"vG{g}") for g in range(4)]
            ones = cpool.tile([128, 128], bf16, tag="ones")
            eye = cpool.tile([128, 128], bf16, tag="eye")
            tri = cpool.tile([128, 128], bf16, tag="tri")

            # ---- projections (RoPE tables live only here) ----
            with tc.tile_pool(name="tables", bufs=1) as tpool, \
                 tc.tile_pool(name="wkp", bufs=1) as wk_pool:
                csd = tpool.tile([128, S], bf16, tag="csd")
                sns = tpool.tile([128, S], bf16, tag="sns")

                def rope(dst, src_ps, tb, rope_pool, st_pool, copy_eng):
                    """Hybrid rope: the half-swapped products read PSUM (the
                    PSUM read port has a partition crossbar; SBUF does not),
                    the rest reads a bf16 SBUF stage so the PSUM bank is
                    released after just the two swap-muls + the stage copy."""
                    tbs = bass.ts(tb, 512)
                    tmp = rope_pool.tile([128, 512], f32, tag="tmp",
                                         name="tmp")
                    st = st_pool.tile([128, 512], bf16, tag="st", name="st")
                    nc.vector.tensor_mul(tmp[0:64, :], src_ps[64:128, :],
                                         sns[0:64, tbs])
                    nc.vector.tensor_mul(tmp[64:128, :], src_ps[0:64, :],
                                         sns[64:128, tbs])
                    copy_eng(st[:], src_ps[:])
                    nc.vector.tensor_mul(dst[:, tbs], st[:], csd[:, tbs])
                    nc.vector.tensor_add(dst[:, tbs], dst[:, tbs], tmp[:])

                with tc.tile_pool(name="xp", bufs=5) as xpool:
                    # ---- Q projection ----
                    with nc.named_scope("proj_q"), \
                         tc.tile_pool(name="wqp", bufs=1) as wq_pool, \
                         tc.tile_pool(name="wvp", bufs=1) as wv_pool, \
                         tc.tile_pool(name="qst", bufs=3) as qst_pool, \
                         tc.tile_pool(name="qps", bufs=2, space="PSUM") as qps_pool, \
                         tc.tile_pool(name="rope", bufs=2) as rope_pool:
                        wq = load_w_grouped(wq_pool, wq_d, "wq")
                        nc.gpsimd.dma_start(csd[:], csd_d[:])
                        nc.gpsimd.dma_start(sns[:], sns_d[:])
                        nc.gpsimd.dma_start(ones[:], ones_d[:])
                        nc.gpsimd.dma_start(eye[:], eye_d[:])
                        nc.gpsimd.dma_start(tri[:], tri_d[:])
                        # preload the Exp table while ACT is idle
                        scratch = qst_pool.tile([128, 2], f32, tag="scr",
                                                name="scr")
                        nc.scalar.activation(
                            scratch[:, 0:1], csd[:, 0:1],
                            mybir.ActivationFunctionType.Exp)
                        wk = wv = wo = None
                        for tb in range(NB):
                            qps = [qps_pool.tile([128, 512], f32, tag=f"q{h}",
                                                 name=f"qps{h}")
                                   for h in range(HL)]
                            for ep in range(NE // 2):  # e-tile pairs
                                x = xpool.tile([128, 2, 512], bf16, tag="x",
                                               name="x")
                                nc.sync.dma_start(
                                    x[:],
                                    xq_d[bass.ds(256 * ep, 256),
                                         bass.ts(tb, 512)]
                                    .rearrange("(e p) t -> p e t", p=128))
                                for e2 in range(2):
                                    et = 2 * ep + e2
                                    for h in range(HL):
                                        nc.tensor.matmul(
                                            qps[h][:],
                                            wslice(wq, et)[:, bass.ts(h, 128)],
                                            x[:, e2], start=(et == 0),
                                            stop=(et == NE - 1))
                            if tb == 1:
                                # deferred weight prefetch on the sync queue,
                                # sequenced between x-tile streams
                                wk = load_w_grouped(wk_pool, wk_d, "wk",
                                                    engine=nc.sync)
                            if tb == 2:
                                wv = load_w_grouped(wv_pool, wv_d, "wv",
                                                    engine=nc.sync)
                            for h in range(HL):
                                rope(qT[h], qps[h][:], tb, rope_pool,
                                     qst_pool, nc.scalar.copy)

                    # ---- K + V projection ----
                    with nc.named_scope("proj_kv"), \
                         tc.tile_pool(name="kst", bufs=3) as kst_pool, \
                         tc.tile_pool(name="kps", bufs=1, space="PSUM") as kps_pool, \
                         tc.tile_pool(name="vps", bufs=1, space="PSUM") as vps_pool, \
                         tc.tile_pool(name="rope2", bufs=2) as rope_pool:
                        for tb in range(NB):
                            kps = [kps_pool.tile([128, 512], f32, tag=f"k{h}",
                                                 name=f"kps{h}")
                                   for h in range(HL)]
                            vps = [vps_pool.tile([128, ND], f32, tag=f"v{sv}",
                                                 name=f"vps{sv}")
                                   for sv in range(4)]
                            for ep in range(NE // 2):
                                x = xpool.tile([128, 2, 512], bf16, tag="x",
                                               name="x")
                                nc.sync.dma_start(
                                    x[:],
                                    xkv_d[bass.ds(256 * ep, 256),
                                          bass.ts(tb, 512)]
                                    .rearrange("(e p) t -> p e t", p=128))
                                for e2 in range(2):
                                    et = 2 * ep + e2
                                    for h in range(HL):
                                        nc.tensor.matmul(
                                            kps[h][:],
                                            wslice(wk, et)[:, bass.ts(h, 128)],
                                            x[:, e2], start=(et == 0),
                                            stop=(et == NE - 1))
                                    for sv in range(4):
                                        nc.tensor.matmul(
                                            vps[sv][:],
                                            x[:, e2, bass.ts(sv, 128)],
                                            wslice(wv, et), start=(et == 0),
                                            stop=(et == NE - 1))
                            if tb == 0:
                                # Wo prefetch on the sync queue (used by the
                                # out-projection interleaved into attention)
                                wo = []
                                for h in range(HL):
                                    w = wo_pool.tile([128, E], bf16,
                                                     tag=f"wo{h}",
                                                     name=f"wo{h}")
                                    nc.sync.dma_start(
                                        w[:], wo_d_bf[bass.ts(h, 128), :])
                                    wo.append(w)
                            for h in range(HL):
                                # stage copy on DVE: keeps the ACT queue free
                                # of backlog at the kv->attn boundary (the
                                # first exps must not queue behind copies)
                                rope(kT[h], kps[h][:], tb, rope_pool,
                                     kst_pool, nc.vector.tensor_copy)
                            for sv in range(4):
                                nc.scalar.copy(vG[tb][:, bass.ts(sv, 512)],
                                               vps[sv][:])

            # ---- Attention + out-projection, software-pipelined ----
            # PSUM pool order matters: up/op land on banks freed by the
            # (slow, DVE-bound) K-rope of the last block; sps/den land on
            # banks freed by the fast scalar V-evictions — so attention can
            # start before the K-rope tail drains.
            with nc.named_scope("attn"), \
                 tc.tile_pool(name="uTp", bufs=1) as ut_pool, \
                 tc.tile_pool(name="et", bufs=1) as et_pool, \
                 tc.tile_pool(name="ups", bufs=2, space="PSUM") as ups_pool, \
                 tc.tile_pool(name="ops", bufs=2, space="PSUM") as ops_pool, \
                 tc.tile_pool(name="sps", bufs=3, space="PSUM") as sps_pool, \
                 tc.tile_pool(name="dps", bufs=1, space="PSUM") as dps_pool, \
                 tc.tile_pool(name="rcp", bufs=2) as rcp_pool, \
                 tc.tile_pool(name="ob", bufs=3) as ob_pool:
                uT = [ut_pool.tile([128, S], bf16, tag=f"uT{h}", name=f"uT{h}")
                      for h in range(HL)]
                # exp tiles, double-buffered across pipeline generations
                eG = [[et_pool.tile([128, 2048], bf16, tag=f"eG{gen}{g}",
                                    name=f"eG{gen}{g}") for g in range(4)]
                      for gen in range(2)]

                def e_ap(gen, si, off=0):
                    base = 512 * (si % 4)
                    return eG[gen][si // 4][:, base + off:base + 512]

                blocks = [(tb, h) for tb in range(NB) for h in range(HL)]

                def live_off(tb, si):
                    """First live column (within the 512-wide t block) of
                    s-tile si; cols below it are fully masked."""
                    v = si - 4 * tb
                    return 128 * v if v > 0 else 0

                def sc_chunks(i):
                    """Scores + mask + exp for block i, one chunk per s-tile."""
                    tb, h = blocks[i]
                    gen = i % 2
                    nsi = 4 * (tb + 1)
                    chunks = []
                    for si in range(nsi):
                        def emit(si=si, tb=tb, h=h, gen=gen):
                            v = si - 4 * tb
                            off = live_off(tb, si)
                            sp = sps_pool.tile([128, 512], f32, tag="sp",
                                               name="sp")
                            nc.tensor.matmul(
                                sp[:, off:512], kT[h][:, bass.ts(si, 128)],
                                qT[h][:, 512 * tb + off:512 * (tb + 1)],
                                start=True, stop=(v < 0))
                            if v >= 0:
                                nc.tensor.matmul(
                                    sp[:, off:off + 128], eye[:], tri[:],
                                    start=False, stop=True)
                            nc.scalar.activation(e_ap(gen, si, off),
                                                 sp[:, off:512], Exp)
                        chunks.append(emit)
                    return chunks

                def da_chunks(i):
                    """Denominator, reciprocal, A@V, normalize for block i."""
                    tb, h = blocks[i]
                    gen = i % 2
                    nsi = 4 * (tb + 1)
                    state = {}

                    def start():
                        state["den"] = dps_pool.tile([128, 512], f32,
                                                     tag="den", name="den")
                        state["up"] = ups_pool.tile([128, 512], f32,
                                                    tag="up", name="up")
                        state["rec"] = rcp_pool.tile([128, 512], f32,
                                                     tag="rec", name="rec")
                    chunks = [start]
                    for si in range(nsi):
                        def emit(si=si, tb=tb, gen=gen):
                            off = live_off(tb, si)
                            nc.tensor.matmul(
                                state["den"][:, off:512], ones[:],
                                e_ap(gen, si, off), start=(si == 0),
                                stop=(si == nsi - 1))
                            if si == nsi - 1:
                                nc.vector.reciprocal_approx_fast(
                                    state["rec"][:], state["den"][:])
                        chunks.append(emit)
                    for si in range(nsi):
                        def emit(si=si, tb=tb, h=h, gen=gen):
                            g, sv = si // 4, si % 4
                            off = live_off(tb, si)
                            nc.tensor.matmul(
                                state["up"][:, off:512],
                                vG[g][:, 512 * sv + 128 * h:
                                      512 * sv + 128 * (h + 1)],
                                e_ap(gen, si, off), start=(si == 0),
                                stop=(si == nsi - 1))
                            if si == nsi - 1:
                                nc.vector.tensor_mul(
                                    uT[h][:, bass.ts(tb, 512)],
                                    state["up"][:], state["rec"][:])
                        chunks.append(emit)
                    return chunks

                def op_chunks(tb):
                    """Out-projection for row block tb (needs uT[*][tb])."""
                    chunks = []
                    for tt in range(4 * tb, 4 * tb + 4):
                        for ec in range(4):
                            def emit(tt=tt, ec=ec):
                                op = ops_pool.tile([128, 512], f32, tag="op",
                                                   name="op")
                                for h in range(HL):
                                    nc.tensor.matmul(
                                        op[:], uT[h][:, bass.ts(tt, 128)],
                                        wo[h][:, bass.ts(ec, 512)],
                                        start=(h == 0), stop=(h == HL - 1))
                                ob = ob_pool.tile([128, 512], f32, tag="ob",
                                                  name="ob")
                                # alternate engines so the DVE FIFO never
                                # backs up in front of the reciprocal
                                if ec % 2 == 0:
                                    nc.vector.tensor_copy(ob[:], op[:])
                                else:
                                    nc.scalar.copy(ob[:], op[:])
                                nc.sync.dma_start(
                                    out_d[bass.ts(tt, 128),
                                          bass.ds(512 * ec, 512)], ob[:])
                            chunks.append(emit)
                    return chunks

                def merge(a, b):
                    na, nb_ = len(a), len(b)
                    ia = ib = 0
                    while ia < na or ib < nb_:
                        if ib >= nb_ or (ia < na and ia * nb_ <= ib * na):
                            a[ia]()
                            ia += 1
                        else:
                            b[ib]()
                            ib += 1

                for i in range(len(blocks)):
                    sc = sc_chunks(i)
                    da = da_chunks(i - 1) if i > 0 else []
                    ptb, ph = blocks[i - 1] if i > 0 else (0, 0)
                    if i > 0 and ph == HL - 1:
                        da = da + op_chunks(ptb)
                    merge(sc, da)
                last = len(blocks) - 1
                for c in da_chunks(last) + op_chunks(blocks[last][0]):
                    c()

    nc.compile()
    return nc


def _get_module():
    if "nc" not in _NC_CACHE:
        _NC_CACHE["nc"] = _build_module()
    return _NC_CACHE["nc"]


def _host_prep(inputs_q, inputs_kv, positions, Wq, Wk, Wv, Wo):
    """Build the 8 per-core input maps."""
    import ml_dtypes
    bf16 = ml_dtypes.bfloat16

    perm = np.concatenate([np.arange(0, D, 2), np.arange(1, D, 2)])  # de-interleave
    scale = np.float32(1.0 / np.sqrt(D))
    half = D // 2
    timescale = 10000.0 ** (2.0 * np.arange(half, dtype=np.float64) / D)
    ones = np.ones((128, 128), dtype=bf16)
    eye = np.eye(128, dtype=np.float32).astype(bf16)
    s_i = np.arange(128)[:, None]
    c_i = np.arange(128)[None, :]
    tri = np.where(c_i < s_i, MASK_VALUE, 0.0).astype(bf16)

    in_maps = []
    for c in range(8):
        b = c // 4
        h0 = (c % 4) * HL
        angle = positions[b].astype(np.float64)[None, :] / timescale[:, None]  # [64,S]
        cs = np.cos(angle).astype(np.float32)
        sn = np.sin(angle).astype(np.float32)
        csd = np.concatenate([cs, cs], axis=0).astype(bf16)      # [128, S]
        sns = np.concatenate([-sn, sn], axis=0).astype(bf16)     # [128, S]
        wq = (Wq[:, h0:h0 + HL, :][:, :, perm] * scale).reshape(E, ND)
        wk = Wk[:, h0:h0 + HL, :][:, :, perm].reshape(E, ND)
        wv = Wv[:, h0:h0 + HL, :].reshape(E, ND)
        wo = Wo[h0:h0 + HL].reshape(ND, E)
        in_maps.append({
            "xq_t": np.ascontiguousarray(inputs_q[b].T).astype(bf16),
            "xkv_t": np.ascontiguousarray(inputs_kv[b].T).astype(bf16),
            "wq": np.ascontiguousarray(wq.astype(bf16)),
            "wk": np.ascontiguousarray(wk.astype(bf16)),
            "wv": np.ascontiguousarray(wv.astype(bf16)),
            "wo": np.ascontiguousarray(wo.astype(bf16)),
            "csd": csd, "sns": sns, "ones": ones, "eye": eye, "tri": tri,
        })
    return in_maps


def kernel(inputs_q, inputs_kv, positions, Wq, Wk, Wv, Wo, _trace=False,
           _trace_kwargs=None):
    from concourse import bass_utils

    nc = _get_module()
    in_maps = _host_prep(inputs_q, inputs_kv, positions, Wq, Wk, Wv, Wo)
    res = bass_utils.run_bass_kernel_spmd(
        nc, in_maps, core_ids=list(range(8)), trace=_trace,
        **(_trace_kwargs or {}))
    if _trace:
        _NC_CACHE["last_results"] = res
    parts = [res.results[c]["out"] for c in range(8)]
    out0 = parts[0] + parts[1] + parts[2] + parts[3]
    out1 = parts[4] + parts[5] + parts[6] + parts[7]
    return np.stack([out0, out1]).astype(np.float32)


# revision 24
# speedup vs baseline: 1.3410x; 1.0187x over previous
"""Multi-head dot-product attention (RoPE, causal) on 8 NeuronCores.

Sharding: data-parallel over batch (2) x tensor-parallel over heads (16 -> 4
per core). Each core projects q/k/v for its 4 heads, runs causal attention,
and computes a partial output projection; the host sums the 4 partials per
batch element.

v2 design notes (vs the f32r baseline):
- All projection / score / denominator / A@V matmuls take bf16 operands
  (same PE rate as f32r at 512-wide, but half the DMA traffic and fast
  weight loads). Out-projection stays f32r (uT stationary, Wo moving).
- Causal mask is applied inside the scores matmul accumulation: a second
  128-wide matmul (identity stationary, triangle-mask moving) adds
  MASK_VALUE over the diagonal 128x128 triangle. Off-diagonal-masked
  columns of diagonal s-tiles are skipped entirely (scores/exp/den/AV all
  run on the live column range only).
- Softmax denominator comes from an all-ones stationary matmul (row sum
  replicated over partitions); 1/den uses reciprocal_approx_fast (~5x
  faster than the exact DVE reciprocal, ~18 bits).
- Attention is software-pipelined with a 1-block skew: PE issues scores of
  block i interleaved with den/AV of block i-1 (exp output double-buffered),
  so the Scalar-engine exp never stalls the PE. Out-projection matmuls for
  a row block are interleaved right after its last head, spreading the
  output DMA across the attention phase.
- RoPE uses a de-interleaved head dim (even dims | odd dims), folded into a
  host-side permutation of Wq/Wk columns; scores are permutation-invariant.
  K-rope runs from a bf16 SBUF stage (PSUM bank freed by a fast scalar
  copy); Q-rope reads PSUM directly (enough banks for double buffering).
"""

import numpy as np

B, S, E, N, D = 2, 2048, 2048, 16, 128
HL = 4           # local heads per core (8 cores = 2 batch x 4 head groups)
ND = HL * D      # 512
NT = S // 128    # 16 row tiles
NB = S // 512    # 4 row blocks
NE = E // 128    # 16 contraction tiles
MASK_VALUE = float(-0.7 * np.finfo(np.float32).max)

_NC_CACHE = {}


def _build_module():
    import concourse.bass as bass
    import concourse.mybir as mybir
    import concourse.tile as tile
    from concourse import bacc

    f32 = mybir.dt.float32
    f32r = mybir.dt.float32r
    bf16 = mybir.dt.bfloat16
    Exp = mybir.ActivationFunctionType.Exp

    nc = bacc.Bacc("TRN2", target_bir_lowering=False, debug=False, num_devices=8)

    xq_d = nc.dram_tensor("xq_t", [E, S], bf16, kind="ExternalInput").ap()
    xkv_d = nc.dram_tensor("xkv_t", [E, S], bf16, kind="ExternalInput").ap()
    wq_d = nc.dram_tensor("wq", [E, ND], bf16, kind="ExternalInput").ap()
    wk_d = nc.dram_tensor("wk", [E, ND], bf16, kind="ExternalInput").ap()
    wv_d = nc.dram_tensor("wv", [E, ND], bf16, kind="ExternalInput").ap()
    wo_d_bf = nc.dram_tensor("wo", [ND, E], bf16, kind="ExternalInput").ap()
    csd_d = nc.dram_tensor("csd", [128, S], bf16, kind="ExternalInput").ap()
    sns_d = nc.dram_tensor("sns", [128, S], bf16, kind="ExternalInput").ap()
    ones_d = nc.dram_tensor("ones", [128, 128], bf16, kind="ExternalInput").ap()
    eye_d = nc.dram_tensor("eye", [128, 128], bf16, kind="ExternalInput").ap()
    tri_d = nc.dram_tensor("tri", [128, 128], bf16, kind="ExternalInput").ap()
    out_d = nc.dram_tensor("out", [S, E], f32, kind="ExternalOutput").ap()

    def load_w_grouped(pool, dram, tag, engine=None):
        """[E, ND] bf16 weights as 4 tiles [128, 4*ND] (4 e-subtiles each)."""
        eng = engine or nc.gpsimd
        ws = []
        for eg in range(4):
            w = pool.tile([128, 4 * ND], bf16, tag=f"{tag}{eg}",
                          name=f"{tag}{eg}")
            eng.dma_start(
                w[:].rearrange("p (e n) -> p e n", e=4),
                dram[bass.ds(512 * eg, 512), :]
                .rearrange("(e p) n -> p e n", p=128))
            ws.append(w)
        return ws

    def wslice(ws, et):
        return ws[et // 4][:, bass.ds(512 * (et % 4), 512)]

    with tile.TileContext(nc) as tc:
        with tc.tile_pool(name="qkp", bufs=1) as qk_pool, \
             tc.tile_pool(name="vp", bufs=1) as v_pool, \
             tc.tile_pool(name="wop", bufs=1) as wo_pool, \
             tc.tile_pool(name="cst", bufs=1) as cpool:
            qT = [qk_pool.tile([128, S], bf16, tag=f"qT{h}", name=f"qT{h}")
                  for h in range(HL)]
            kT = [qk_pool.tile([128, S], bf16, tag=f"kT{h}", name=f"kT{h}")
                  for h in range(HL)]
            vG = [v_pool.tile([128, 4 * ND], bf16, tag=f"vG{g}",
                              name=f"vG{g}") for g in range(4)]
            ones = cpool.tile([128, 128], bf16, tag="ones")
            eye = cpool.tile([128, 128], bf16, tag="eye")
            tri = cpool.tile([128, 128], bf16, tag="tri")

            # ---- projections (RoPE tables live only here) ----
            with tc.tile_pool(name="tables", bufs=1) as tpool, \
                 tc.tile_pool(name="wkp", bufs=1) as wk_pool:
                csd = tpool.tile([128, S], bf16, tag="csd")
                sns = tpool.tile([128, S], bf16, tag="sns")

                def rope(dst, src_ps, tb, rope_pool, st_pool, copy_eng):
                    """Staged rope: ONE stage copy is the only PSUM reader
                    (bank freed in ~0.6us); the partition half-swap runs on
                    the idle DMA engines; DVE does 3 aligned bf16 ops."""
                    tbs = bass.ts(tb, 512)
                    tmp = rope_pool.tile([128, 512], f32, tag="tmp",
                                         name="tmp")
                    st = st_pool.tile([128, 512], bf16, tag="st", name="st")
                    sw = st_pool.tile([128, 512], bf16, tag="sw", name="sw")
                    copy_eng(st[:], src_ps[:])
                    nc.gpsimd.dma_start(sw[0:64, :], st[64:128, :])
                    nc.gpsimd.dma_start(sw[64:128, :], st[0:64, :])
                    nc.vector.tensor_mul(tmp[:], sw[:], sns[:, tbs])
                    nc.vector.tensor_mul(dst[:, tbs], st[:], csd[:, tbs])
                    nc.vector.tensor_add(dst[:, tbs], dst[:, tbs], tmp[:])

                with tc.tile_pool(name="xp", bufs=5) as xpool:
                    # ---- Q projection ----
                    with nc.named_scope("proj_q"), \
                         tc.tile_pool(name="wqp", bufs=1) as wq_pool, \
                         tc.tile_pool(name="wvp", bufs=1) as wv_pool, \
                         tc.tile_pool(name="qst", bufs=3) as qst_pool, \
                         tc.tile_pool(name="qps", bufs=2, space="PSUM") as qps_pool, \
                         tc.tile_pool(name="rope", bufs=2) as rope_pool:
                        wq = load_w_grouped(wq_pool, wq_d, "wq")
                        nc.gpsimd.dma_start(csd[:], csd_d[:])
                        nc.gpsimd.dma_start(sns[:], sns_d[:])
                        nc.gpsimd.dma_start(ones[:], ones_d[:])
                        nc.gpsimd.dma_start(eye[:], eye_d[:])
                        nc.gpsimd.dma_start(tri[:], tri_d[:])
                        # preload the Exp table while ACT is idle
                        scratch = qst_pool.tile([128, 2], f32, tag="scr",
                                                name="scr")
                        nc.scalar.activation(
                            scratch[:, 0:1], csd[:, 0:1],
                            mybir.ActivationFunctionType.Exp)
                        wk = wv = wo = None
                        for tb in range(NB):
                            qps = [qps_pool.tile([128, 512], f32, tag=f"q{h}",
                                                 name=f"qps{h}")
                                   for h in range(HL)]
                            for ep in range(NE // 4):  # groups of 4 e-tiles
                                x = xpool.tile([128, 4, 512], bf16, tag="x",
                                               name="x")
                                nc.sync.dma_start(
                                    x[:],
                                    xq_d[bass.ds(512 * ep, 512),
                                         bass.ts(tb, 512)]
                                    .rearrange("(e p) t -> p e t", p=128))
                                for e2 in range(4):
                                    et = 4 * ep + e2
                                    for h in range(HL):
                                        nc.tensor.matmul(
                                            qps[h][:],
                                            wslice(wq, et)[:, bass.ts(h, 128)],
                                            x[:, e2], start=(et == 0),
                                            stop=(et == NE - 1))
                            if tb == 1:
                                # deferred weight prefetch on the sync queue,
                                # sequenced between x-tile streams
                                wk = load_w_grouped(wk_pool, wk_d, "wk",
                                                    engine=nc.sync)
                            if tb == 2:
                                wv = load_w_grouped(wv_pool, wv_d, "wv",
                                                    engine=nc.sync)
                            for h in range(HL):
                                rope(qT[h], qps[h][:], tb, rope_pool,
                                     qst_pool, nc.scalar.copy)

                    # ---- K + V projection ----
                    with nc.named_scope("proj_kv"), \
                         tc.tile_pool(name="kst", bufs=3) as kst_pool, \
                         tc.tile_pool(name="kps", bufs=1, space="PSUM") as kps_pool, \
                         tc.tile_pool(name="vps", bufs=1, space="PSUM") as vps_pool, \
                         tc.tile_pool(name="rope2", bufs=2) as rope_pool:
                        for tb in range(NB):
                            kps = [kps_pool.tile([128, 512], f32, tag=f"k{h}",
                                                 name=f"kps{h}")
                                   for h in range(HL)]
                            vps = [vps_pool.tile([128, ND], f32, tag=f"v{sv}",
                                                 name=f"vps{sv}")
                                   for sv in range(4)]
                            for ep in range(NE // 4):
                                x = xpool.tile([128, 4, 512], bf16, tag="x",
                                               name="x")
                                nc.sync.dma_start(
                                    x[:],
                                    xkv_d[bass.ds(512 * ep, 512),
                                          bass.ts(tb, 512)]
                                    .rearrange("(e p) t -> p e t", p=128))
                                for e2 in range(4):
                                    et = 4 * ep + e2
                                    for h in range(HL):
                                        nc.tensor.matmul(
                                            kps[h][:],
                                            wslice(wk, et)[:, bass.ts(h, 128)],
                                            x[:, e2], start=(et == 0),
                                            stop=(et == NE - 1))
                                    for sv in range(4):
                                        nc.tensor.matmul(
                                            vps[sv][:],
                                            x[:, e2, bass.ts(sv, 128)],
                                            wslice(wv, et), start=(et == 0),
                                            stop=(et == NE - 1))
                            if tb == 0:
                                # Wo prefetch on the sync queue (used by the
                                # out-projection interleaved into attention)
                                wo = []
                                for h in range(HL):
                                    w = wo_pool.tile([128, E], bf16,
                                                     tag=f"wo{h}",
                                                     name=f"wo{h}")
                                    nc.sync.dma_start(
                                        w[:], wo_d_bf[bass.ts(h, 128), :])
                                    wo.append(w)
                            for h in range(HL):
                                # last block: stage on DVE so the ACT queue
                                # is clear for attention's first exps
                                ceng = (nc.vector.tensor_copy if tb == NB - 1
                                        else nc.scalar.copy)
                                rope(kT[h], kps[h][:], tb, rope_pool,
                                     kst_pool, ceng)
                            for sv in range(4):
                                nc.scalar.copy(vG[tb][:, bass.ts(sv, 512)],
                                               vps[sv][:])

            # ---- Attention + out-projection, software-pipelined ----
            # PSUM pool order matters: up/op land on banks freed by the
            # (slow, DVE-bound) K-rope of the last block; sps/den land on
            # banks freed by the fast scalar V-evictions — so attention can
            # start before the K-rope tail drains.
            with nc.named_scope("attn"), \
                 tc.tile_pool(name="uTp", bufs=1) as ut_pool, \
                 tc.tile_pool(name="et", bufs=1) as et_pool, \
                 tc.tile_pool(name="sps", bufs=4, space="PSUM") as sps_pool, \
                 tc.tile_pool(name="dps", bufs=1, space="PSUM") as dps_pool, \
                 tc.tile_pool(name="ups", bufs=1, space="PSUM") as ups_pool, \
                 tc.tile_pool(name="ops", bufs=2, space="PSUM") as ops_pool, \
                 tc.tile_pool(name="rcp", bufs=2) as rcp_pool, \
                 tc.tile_pool(name="ob", bufs=3) as ob_pool:
                uT = [ut_pool.tile([128, S], bf16, tag=f"uT{h}", name=f"uT{h}")
                      for h in range(HL)]
                # exp tiles, double-buffered across pipeline generations
                eG = [[et_pool.tile([128, 2048], bf16, tag=f"eG{gen}{g}",
                                    name=f"eG{gen}{g}") for g in range(4)]
                      for gen in range(2)]

                def e_ap(gen, si, off=0):
                    base = 512 * (si % 4)
                    return eG[gen][si // 4][:, base + off:base + 512]

                blocks = [(tb, h) for tb in range(NB) for h in range(HL)]

                def live_off(tb, si):
                    """First live column (within the 512-wide t block) of
                    s-tile si; cols below it are fully masked."""
                    v = si - 4 * tb
                    return 128 * v if v > 0 else 0

                def sc_chunks(i):
                    """Scores + mask + exp for block i, one chunk per s-tile."""
                    tb, h = blocks[i]
                    gen = i % 2
                    nsi = 4 * (tb + 1)
                    chunks = []
                    for si in range(nsi):
                        def emit(si=si, tb=tb, h=h, gen=gen):
                            v = si - 4 * tb
                            off = live_off(tb, si)
                            sp = sps_pool.tile([128, 512], f32, tag="sp",
                                               name="sp")
                            nc.tensor.matmul(
                                sp[:, off:512], kT[h][:, bass.ts(si, 128)],
                                qT[h][:, 512 * tb + off:512 * (tb + 1)],
                                start=True, stop=(v < 0))
                            if v >= 0:
                                nc.tensor.matmul(
                                    sp[:, off:off + 128], eye[:], tri[:],
                                    start=False, stop=True)
                            nc.scalar.activation(e_ap(gen, si, off),
                                                 sp[:, off:512], Exp)
                        chunks.append(emit)
                    return chunks

                def da_chunks(i):
                    """Denominator, reciprocal, A@V, normalize for block i."""
                    tb, h = blocks[i]
                    gen = i % 2
                    nsi = 4 * (tb + 1)
                    state = {}

                    def start():
                        state["den"] = dps_pool.tile([128, 512], f32,
                                                     tag="den", name="den")
                        state["up"] = ups_pool.tile([128, 512], f32,
                                                    tag="up", name="up")
                        state["rec"] = rcp_pool.tile([128, 512], f32,
                                                     tag="rec", name="rec")
                    chunks = [start]
                    for si in range(nsi):
                        def emit(si=si, tb=tb, gen=gen):
                            off = live_off(tb, si)
                            nc.tensor.matmul(
                                state["den"][:, off:512], ones[:],
                                e_ap(gen, si, off), start=(si == 0),
                                stop=(si == nsi - 1))
                            if si == nsi - 1:
                                nc.vector.reciprocal_approx_fast(
                                    state["rec"][:], state["den"][:])
                        chunks.append(emit)
                    for si in range(nsi):
                        def emit(si=si, tb=tb, h=h, gen=gen):
                            g, sv = si // 4, si % 4
                            off = live_off(tb, si)
                            nc.tensor.matmul(
                                state["up"][:, off:512],
                                vG[g][:, 512 * sv + 128 * h:
                                      512 * sv + 128 * (h + 1)],
                                e_ap(gen, si, off), start=(si == 0),
                                stop=(si == nsi - 1))
                            if si == nsi - 1:
                                nc.vector.tensor_mul(
                                    uT[h][:, bass.ts(tb, 512)],
                                    state["up"][:], state["rec"][:])
                        chunks.append(emit)
                    return chunks

                def op_chunks(tb):
                    """Out-projection for row block tb (needs uT[*][tb])."""
                    chunks = []
                    for tt in range(4 * tb, 4 * tb + 4):
                        for ec in range(4):
                            def emit(tt=tt, ec=ec):
                                op = ops_pool.tile([128, 512], f32, tag="op",
                                                   name="op")
                                for h in range(HL):
                                    nc.tensor.matmul(
                                        op[:], uT[h][:, bass.ts(tt, 128)],
                                        wo[h][:, bass.ts(ec, 512)],
                                        start=(h == 0), stop=(h == HL - 1))
                                ob = ob_pool.tile([128, 512], f32, tag="ob",
                                                  name="ob")
                                # alternate engines so the DVE FIFO never
                                # backs up in front of the reciprocal
                                if ec % 2 == 0:
                                    nc.vector.tensor_copy(ob[:], op[:])
                                else:
                                    nc.scalar.copy(ob[:], op[:])
                                nc.sync.dma_start(
                                    out_d[bass.ts(tt, 128),
                                          bass.ds(512 * ec, 512)], ob[:])
                            chunks.append(emit)
                    return chunks

                def merge(a, b):
                    na, nb_ = len(a), len(b)
                    ia = ib = 0
                    while ia < na or ib < nb_:
                        if ib >= nb_ or (ia < na and ia * nb_ <= ib * na):
                            a[ia]()
                            ia += 1
                        else:
                            b[ib]()
                            ib += 1

                for i in range(len(blocks)):
                    sc = sc_chunks(i)
                    da = da_chunks(i - 1) if i > 0 else []
                    ptb, ph = blocks[i - 1] if i > 0 else (0, 0)
                    if i > 0 and ph == HL - 1:
                        da = da + op_chunks(ptb)
                    merge(sc, da)
                last = len(blocks) - 1
                for c in da_chunks(last) + op_chunks(blocks[last][0]):
                    c()

    nc.compile()
    return nc


def _get_module():
    if "nc" not in _NC_CACHE:
        _NC_CACHE["nc"] = _build_module()
    return _NC_CACHE["nc"]


def _host_prep(inputs_q, inputs_kv, positions, Wq, Wk, Wv, Wo):
    """Build the 8 per-core input maps."""
    import ml_dtypes
    bf16 = ml_dtypes.bfloat16

    perm = np.concatenate([np.arange(0, D, 2), np.arange(1, D, 2)])  # de-interleave
    scale = np.float32(1.0 / np.sqrt(D))
    half = D // 2
    timescale = 10000.0 ** (2.0 * np.arange(half, dtype=np.float64) / D)
    ones = np.ones((128, 128), dtype=bf16)
    eye = np.eye(128, dtype=np.float32).astype(bf16)
    s_i = np.arange(128)[:, None]
    c_i = np.arange(128)[None, :]
    tri = np.where(c_i < s_i, MASK_VALUE, 0.0).astype(bf16)

    in_maps = []
    for c in range(8):
        b = c // 4
        h0 = (c % 4) * HL
        angle = positions[b].astype(np.float64)[None, :] / timescale[:, None]  # [64,S]
        cs = np.cos(angle).astype(np.float32)
        sn = np.sin(angle).astype(np.float32)
        csd = np.concatenate([cs, cs], axis=0).astype(bf16)      # [128, S]
        sns = np.concatenate([-sn, sn], axis=0).astype(bf16)     # [128, S]
        wq = (Wq[:, h0:h0 + HL, :][:, :, perm] * scale).reshape(E, ND)
        wk = Wk[:, h0:h0 + HL, :][:, :, perm].reshape(E, ND)
        wv = Wv[:, h0:h0 + HL, :].reshape(E, ND)
        wo = Wo[h0:h0 + HL].reshape(ND, E)
        in_maps.append({
            "xq_t": np.ascontiguousarray(inputs_q[b].T).astype(bf16),
            "xkv_t": np.ascontiguousarray(inputs_kv[b].T).astype(bf16),
            "wq": np.ascontiguousarray(wq.astype(bf16)),
            "wk": np.ascontiguousarray(wk.astype(bf16)),
            "wv": np.ascontiguousarray(wv.astype(bf16)),
            "wo": np.ascontiguousarray(wo.astype(bf16)),
            "csd": csd, "sns": sns, "ones": ones, "eye": eye, "tri": tri,
        })
    return in_maps


def kernel(inputs_q, inputs_kv, positions, Wq, Wk, Wv, Wo, _trace=False,
           _trace_kwargs=None):
    from concourse import bass_utils

    nc = _get_module()
    in_maps = _host_prep(inputs_q, inputs_kv, positions, Wq, Wk, Wv, Wo)
    res = bass_utils.run_bass_kernel_spmd(
        nc, in_maps, core_ids=list(range(8)), trace=_trace,
        **(_trace_kwargs or {}))
    if _trace:
        _NC_CACHE["last_results"] = res
    parts = [res.results[c]["out"] for c in range(8)]
    out0 = parts[0] + parts[1] + parts[2] + parts[3]
    out1 = parts[4] + parts[5] + parts[6] + parts[7]
    return np.stack([out0, out1]).astype(np.float32)
